# revision 17
# baseline (speedup 1.0000x reference)
"""HINormer sparse-attention kernel for Trainium2 (8 NeuronCores, SPMD).

Math (reference reformulated):
  softmax_t(sl[s] + sr[t] + bil[s,t]) == softmax_t(sr[t] + bil[s,t])
    -> the whole fl = h@Wl / al branch cancels (constant per softmax row).
  Softmax denominator rides the context matmul as an appended ones-column.

Sharding: core c -> (batch b = c//2, query-half q = c%2). Each core computes
complete output rows LN(h + fh) for its 1024 query rows; no collectives.

Per-core dataflow (all matmuls bf16, PSUM fp32):
  S1: fr[t, d'] = hT.T @ Wr  (d' = (head, hd) col blocks); from PSUM:
      - cast-copy into frp[t, 65-col blocks] (bf16, ones col per head)
      - leaky = (fr*0.01) max fr; sr_all[t-tile, head] = sum_hd leaky*ar
  S2: rq[hd_pair, t] = Wrt_pair.T @ rhT ; rkT[hd_pair, s_q] = Wrs_pair.T @ rhTq
  S3 per head: L_T[t, s] = rq_h.T @ rkT_h  (K=64); P = exp(L_T + sr bias)
      ctx.T[hd+1, s] = frp_h.T @ P_T  (K=t, 16 tiles; row 64 = denominator)
  S4: h_saT = ctx.T * (1/denom broadcast); fh[s,:] = h_saT.T @ Wf; LN + out.
"""

import sys

for _p in ("/opt/trn_rl_repo",):
    if _p not in sys.path:
        sys.path.append(_p)

import numpy as np
import ml_dtypes

BF16 = ml_dtypes.bfloat16

B, S, D = 4, 2048, 512
H, HD, RL = 8, 64, 64
SLOPE = 0.01
LN_EPS = 1e-5
NCORES = 8
SQ = S // 2          # 1024 query rows per core
KT = S // 128        # 16 key/t tiles
MQ = SQ // 128       # 8 query s-tiles
NCH = SQ // 512      # 2 512-chunks of the query dim
DK = D // 128        # 4 d-tiles

_CACHE = {}


def _build():
    import concourse.bacc as bacc
    import concourse.tile as tile
    import concourse.bass as bass
    from concourse import mybir

    f32 = mybir.dt.float32
    bf16 = mybir.dt.bfloat16
    Exp = mybir.ActivationFunctionType.Exp
    Sqrt = mybir.ActivationFunctionType.Sqrt
    Alu = mybir.AluOpType
    AxX = mybir.AxisListType.X

    nc = bacc.Bacc("TRN2", target_bir_lowering=False, debug=False,
                   num_devices=NCORES)

    def din(name, shape, dt):
        return nc.dram_tensor(name, shape, dt, kind="ExternalInput").ap()

    hT = din("hT", [D, S], bf16)          # h[b].T
    hrows = din("hrows", [SQ, D], f32)    # h[b, s_rows] (residual, fp32)
    rhT = din("rhT", [RL, S], bf16)       # rh[b].T
    rhTq = din("rhTq", [RL, SQ], bf16)    # rh[b, s_rows].T
    Wr_d = din("Wr", [D, D], bf16)
    Wrs_d = din("Wrs", [RL, D], bf16)     # cols already head-major
    Wrt_d = din("Wrt", [RL, D], bf16)
    Wf_d = din("Wf", [D, D], bf16)
    arv = din("arv", [D], f32)            # ar tiled per head
    g_d = din("g", [D], f32)
    b_d = din("b", [D], f32)
    out = nc.dram_tensor("out", [SQ, D], f32, kind="ExternalOutput").ap()
    rec_dram = nc.dram_tensor("rec_scratch", [H, SQ], f32)  # Internal
    den_dram = nc.dram_tensor("den_scratch", [H, SQ], f32)  # Internal

    def bcast_ap(src_ap, parts, free):
        return bass.AP(tensor=src_ap.tensor, offset=src_ap.offset,
                       ap=[[0, parts], [1, free]])

    with tile.TileContext(nc) as tc:
        ex = tc.nc  # same nc
        with tc.tile_pool(name="singles", bufs=1) as singles:
            # ---- constants / weights ----
            Wr_sb = singles.tile([128, DK, D], bf16)
            nc.sync.dma_start(out=Wr_sb,
                              in_=Wr_d.rearrange("(k p) n -> p k n", p=128))
            Wf_sb = singles.tile([128, DK, D], bf16)
            nc.sync.dma_start(out=Wf_sb,
                              in_=Wf_d.rearrange("(k p) n -> p k n", p=128))
            Wrs_sb = singles.tile([RL, D], bf16)
            nc.sync.dma_start(out=Wrs_sb, in_=Wrs_d)
            Wrt_sb = singles.tile([RL, D], bf16)
            nc.sync.dma_start(out=Wrt_sb, in_=Wrt_d)
            ar_bc = singles.tile([128, D], f32)
            nc.gpsimd.dma_start(out=ar_bc, in_=bcast_ap(arv, 128, D))
            g_bc = singles.tile([128, D], f32)
            nc.gpsimd.dma_start(out=g_bc, in_=bcast_ap(g_d, 128, D))
            b_bc = singles.tile([128, D], f32)
            nc.gpsimd.dma_start(out=b_bc, in_=bcast_ap(b_d, 128, D))
            eps_t = singles.tile([128, 1], f32)
            nc.vector.memset(eps_t, LN_EPS)

            rhT_sb = singles.tile([RL, S], bf16)
            nc.sync.dma_start(out=rhT_sb, in_=rhT)
            rhTq_sb = singles.tile([RL, SQ], bf16)
            nc.sync.dma_start(out=rhTq_sb, in_=rhTq)

            # frp: [t-tile, ti, (head: 64 fr cols + ones col)] bf16
            frp = singles.tile([128, KT, H * (HD + 1)], bf16)
            for h in range(H):
                nc.vector.memset(frp[:, :, h * (HD + 1) + HD:h * (HD + 1) + HD + 1], 1.0)
            sr_all = singles.tile([128, KT, H], f32)

            hT_v = hT.rearrange("(k p) t -> k p t", p=128)

            # ============ S2 first: rq / rkT (feeds S3 earliest) ============
            with tc.tile_pool(name="hTp", bufs=DK) as hTp, \
                 tc.tile_pool(name="s1tmp", bufs=3) as s1tmp:
                ps_a_cm = tc.tile_pool(name="ps_a", bufs=2, space="PSUM")
                ps_a = ps_a_cm.__enter__()
                rq_sb, rkT_sb = [None] * (H // 2), [None] * (H // 2)

                def emit_pair(j):
                    rq = hTp.tile([128, S], bf16, tag="rq", name=f"rq{j}")
                    for n in range(S // 512):
                        ps = ps_a.tile([128, 512], f32, tag="ps", name="ps")
                        nc.tensor.matmul(ps, lhsT=Wrt_sb[:, 128 * j:128 * (j + 1)],
                                         rhs=rhT_sb[:, 512 * n:512 * (n + 1)],
                                         start=True, stop=True)
                        nc.scalar.copy(out=rq[:, 512 * n:512 * (n + 1)], in_=ps)
                    rq_sb[j] = rq
                    rk = hTp.tile([128, SQ], bf16, tag="rk", name=f"rk{j}")
                    for n in range(NCH):
                        ps = ps_a.tile([128, 512], f32, tag="ps", name="ps")
                        nc.tensor.matmul(ps, lhsT=Wrs_sb[:, 128 * j:128 * (j + 1)],
                                         rhs=rhTq_sb[:, 512 * n:512 * (n + 1)],
                                         start=True, stop=True)
                        nc.scalar.copy(out=rk[:, 512 * n:512 * (n + 1)], in_=ps)
                    rkT_sb[j] = rk

                for j in range(H // 2):
                    emit_pair(j)

                # ================= S1: fr projection, frp, sr =================
                hT_sb = []
                for k in range(DK):
                    t = hTp.tile([128, S], bf16, tag="hT")
                    nc.sync.dma_start(out=t, in_=hT_v[k])
                    hT_sb.append(t)
                for i in range(KT):
                    ps = ps_a.tile([128, 512], f32, tag="ps")
                    for k in range(DK):
                        nc.tensor.matmul(ps, lhsT=hT_sb[k][:, 128 * i:128 * (i + 1)],
                                         rhs=Wr_sb[:, k, :],
                                         start=(k == 0), stop=(k == DK - 1))
                    nc.scalar.copy(
                        out=frp[:, i, :].rearrange("p (h c) -> p h c", c=HD + 1)[:, :, 0:HD],
                        in_=ps.rearrange("p (h c) -> p h c", c=HD))
                    lk = s1tmp.tile([128, 8, HD], f32, tag="lk")
                    frp_i = frp[:, i, :].rearrange("p (h c) -> p h c", c=HD + 1)[:, :, 0:HD]
                    nc.vector.scalar_tensor_tensor(
                        out=lk, in0=frp_i, scalar=SLOPE, in1=frp_i,
                        op0=Alu.mult, op1=Alu.max)
                    lka = s1tmp.tile([128, 8, HD], f32, tag="lka")
                    nc.vector.tensor_mul(lka, lk,
                                         ar_bc.rearrange("p (h c) -> p h c", c=HD))
                    nc.vector.reduce_sum(out=sr_all[:, i, :], in_=lka, axis=AxX)

                ps_a_cm.__exit__(None, None, None)

                # ===== S3: per-head attention; normalize folded in per pair =====
                ctx_sb = []
                for j in range(DK):
                    ctile = singles.tile([128, SQ], f32, tag=f"ctx{j}")
                    ctx_sb.append(ctile)
                hsa_sb = [None] * DK
                with tc.tile_pool(name="ps_bil", bufs=3, space="PSUM") as ps_bil, \
                     tc.tile_pool(name="ps_ctx", bufs=2, space="PSUM") as ps_ctx, \
                     tc.tile_pool(name="dstage", bufs=3) as dstage, \
                     tc.tile_pool(name="dpp", bufs=2) as dpp, \
                     tc.tile_pool(name="recb", bufs=2) as recbp, \
                     tc.tile_pool(name="pp", bufs=17) as pp:
                    for h in range(H):
                        j, off = h // 2, 64 * (h % 2)
                        ptiles = []
                        for ti in range(KT):
                            psb = ps_bil.tile([128, SQ], f32, tag="bil")
                            for c in range(NCH):
                                nc.tensor.matmul(
                                    psb[:, 512 * c:512 * (c + 1)],
                                    lhsT=rq_sb[j][off:off + 64, 128 * ti:128 * (ti + 1)],
                                    rhs=rkT_sb[j][off:off + 64, 512 * c:512 * (c + 1)],
                                    start=True, stop=True)
                            pt = pp.tile([128, SQ], bf16, tag="pt")
                            nc.scalar.activation(out=pt, in_=psb, func=Exp,
                                                 bias=sr_all[:, ti, h:h + 1])
                            ptiles.append(pt)
                        for c in range(NCH):
                            psc = ps_ctx.tile([HD + 1, 512], f32, tag="ctxps")
                            for ti in range(KT):
                                nc.tensor.matmul(
                                    psc,
                                    lhsT=frp[:, ti, (HD + 1) * h:(HD + 1) * (h + 1)],
                                    rhs=ptiles[ti][:, 512 * c:512 * (c + 1)],
                                    start=(ti == 0), stop=(ti == KT - 1))
                            nc.vector.tensor_copy(
                                out=ctx_sb[j][off:off + 64, 512 * c:512 * (c + 1)],
                                in_=psc[0:HD, :])
                            dst = dstage.tile([1, 512], f32, tag="dst")
                            nc.vector.tensor_copy(out=dst, in_=psc[HD:HD + 1, :])
                            nc.sync.dma_start(
                                out=den_dram.ap()[h, 512 * c:512 * (c + 1)][None, :],
                                in_=dst)
                        if h % 2 == 1:
                            # normalize this d-tile pair inside the S3 shadow
                            dp = dpp.tile([8, 256], f32, tag="dp")
                            nc.sync.dma_start(
                                out=dp,
                                in_=den_dram.ap().rearrange("a q -> (a q)")[
                                    2 * j * SQ:(2 * j + 2) * SQ].rearrange(
                                        "(p c) -> p c", p=8))
                            rp = dpp.tile([8, 256], f32, tag="rp")
                            nc.vector.reciprocal(rp, dp)
                            nc.sync.dma_start(
                                out=rec_dram.ap().rearrange("a q -> (a q)")[
                                    2 * j * SQ:(2 * j + 2) * SQ].rearrange(
                                        "(p c) -> p c", p=8),
                                in_=rp)
                            recb = recbp.tile([128, SQ], f32, tag="recb")
                            for u in range(2):
                                nc.gpsimd.dma_start(
                                    out=recb[64 * u:64 * (u + 1), :],
                                    in_=bass.AP(tensor=rec_dram,
                                                offset=(2 * j + u) * SQ,
                                                ap=[[0, 64], [1, SQ]]))
                            hsa = hTp.tile([128, SQ], bf16, tag="hT")
                            nc.vector.tensor_mul(hsa, ctx_sb[j], recb)
                            hsa_sb[j] = hsa

                # ================= S4: fh + LN =================
                hrows_v = hrows.rearrange("(m p) d -> m p d", p=128)
                out_v = out.rearrange("(m p) d -> m p d", p=128)
                with tc.tile_pool(name="ps_fh", bufs=2, space="PSUM") as ps_fh, \
                     tc.tile_pool(name="lnp", bufs=6) as lnp, \
                     tc.tile_pool(name="hrp", bufs=3) as hrp:
                    for mi in range(MQ):
                        psf = ps_fh.tile([128, 512], f32, tag="fh")
                        for k in range(DK):
                            nc.tensor.matmul(psf,
                                             lhsT=hsa_sb[k][:, 128 * mi:128 * (mi + 1)],
                                             rhs=Wf_sb[:, k, :],
                                             start=(k == 0), stop=(k == DK - 1))
                        hr = hrp.tile([128, D], f32, tag="hr")
                        nc.sync.dma_start(out=hr, in_=hrows_v[mi])
                        xs = lnp.tile([128, D], f32, tag="xs")
                        nc.vector.tensor_add(xs, psf, hr)
                        stats = lnp.tile([128, 6], f32, tag="stats")
                        nc.vector.bn_stats(stats, xs)
                        mv = lnp.tile([128, 2], f32, tag="mv")
                        nc.vector.bn_aggr(mv, stats)
                        std = lnp.tile([128, 1], f32, tag="std")
                        nc.scalar.activation(out=std, in_=mv[:, 1:2], func=Sqrt,
                                             bias=eps_t)
                        rstd = lnp.tile([128, 1], f32, tag="rstd")
                        nc.vector.reciprocal(rstd, std)
                        xn = lnp.tile([128, D], f32, tag="xn")
                        nc.vector.tensor_scalar(out=xn, in0=xs,
                                                scalar1=mv[:, 0:1], scalar2=rstd,
                                                op0=Alu.subtract, op1=Alu.mult)
                        xo = lnp.tile([128, D], f32, tag="xo")
                        nc.vector.scalar_tensor_tensor(out=xo, in0=xn, scalar=1.0,
                                                       in1=g_bc, op0=Alu.mult,
                                                       op1=Alu.mult)
                        nc.vector.tensor_add(xo, xo, b_bc)
                        nc.sync.dma_start(out=out_v[mi], in_=xo)

    nc.compile()
    return nc


def _get_nc():
    if "nc" not in _CACHE:
        _CACHE["nc"] = _build()
    return _CACHE["nc"]


def _in_maps(h, rh, Wr, ar, Wrs, Wrt, Wf, ln_g, ln_b):
    h = np.asarray(h, np.float32)
    rh = np.asarray(rh, np.float32)
    in_maps = []
    for c in range(NCORES):
        b, q = c // 2, c % 2
        sl = slice(q * SQ, (q + 1) * SQ)
        in_maps.append({
            "hT": np.ascontiguousarray(h[b].T).astype(BF16),
            "hrows": np.ascontiguousarray(h[b, sl]),
            "rhT": np.ascontiguousarray(rh[b].T).astype(BF16),
            "rhTq": np.ascontiguousarray(rh[b, sl].T).astype(BF16),
            "Wr": np.asarray(Wr, np.float32).astype(BF16),
            "Wrs": np.asarray(Wrs, np.float32).astype(BF16),
            "Wrt": np.asarray(Wrt, np.float32).astype(BF16),
            "Wf": np.asarray(Wf, np.float32).astype(BF16),
            "arv": np.ascontiguousarray(np.tile(np.asarray(ar, np.float32), H)),
            "g": np.asarray(ln_g, np.float32),
            "b": np.asarray(ln_b, np.float32),
        })
    return in_maps


def _assemble(results):
    outp = np.empty((B, S, D), np.float32)
    for c in range(NCORES):
        b, q = c // 2, c % 2
        outp[b, q * SQ:(q + 1) * SQ] = results[c]["out"]
    return outp


def kernel(h, rh, Wl, Wr, al, ar, Wrs, Wrt, Wf, ln_g, ln_b, **_ignored):
    nc = _get_nc()
    from concourse.bass_utils import run_bass_kernel_spmd

    in_maps = _in_maps(h, rh, Wr, ar, Wrs, Wrt, Wf, ln_g, ln_b)
    res = run_bass_kernel_spmd(nc, in_maps, core_ids=list(range(NCORES)))
    _CACHE["last_results"] = res
    return _assemble(res.results)


# revision 23
# speedup vs baseline: 8745.6362x; 8745.6362x over previous
"""HINormer sparse-attention kernel for Trainium2 (8 NeuronCores, SPMD).

Math (reference reformulated):
  softmax_t(sl[s] + sr[t] + bil[s,t]) == softmax_t(sr[t] + bil[s,t])
    -> the whole fl = h@Wl / al branch cancels (constant per softmax row).
  Softmax denominator rides the context matmul as an appended ones-column.

Sharding: core c -> (batch b = c//2, query-half q = c%2). Each core computes
complete output rows LN(h + fh) for its 1024 query rows; no collectives.

Per-core dataflow (all matmuls bf16, PSUM fp32):
  S1: fr[t, d'] = hT.T @ Wr  (d' = (head, hd) col blocks); from PSUM:
      - cast-copy into frp[t, 65-col blocks] (bf16, ones col per head)
      - leaky = (fr*0.01) max fr; sr_all[t-tile, head] = sum_hd leaky*ar
  S2: rq[hd_pair, t] = Wrt_pair.T @ rhT ; rkT[hd_pair, s_q] = Wrs_pair.T @ rhTq
  S3 per head: L_T[t, s] = rq_h.T @ rkT_h  (K=64); P = exp(L_T + sr bias)
      ctx.T[hd+1, s] = frp_h.T @ P_T  (K=t, 16 tiles; row 64 = denominator)
  S4: h_saT = ctx.T * (1/denom broadcast); fh[s,:] = h_saT.T @ Wf; LN + out.
"""

import sys

for _p in ("/opt/trn_rl_repo",):
    if _p not in sys.path:
        sys.path.append(_p)

import numpy as np
import ml_dtypes

BF16 = ml_dtypes.bfloat16

B, S, D = 4, 2048, 512
H, HD, RL = 8, 64, 64
SLOPE = 0.01
LN_EPS = 1e-5
NCORES = 8
SQ = S // 2          # 1024 query rows per core
KT = S // 128        # 16 key/t tiles
MQ = SQ // 128       # 8 query s-tiles
NCH = SQ // 512      # 2 512-chunks of the query dim
DK = D // 128        # 4 d-tiles

_CACHE = {}


def _build():
    import concourse.bacc as bacc
    import concourse.tile as tile
    import concourse.bass as bass
    from concourse import mybir

    f32 = mybir.dt.float32
    bf16 = mybir.dt.bfloat16
    Exp = mybir.ActivationFunctionType.Exp
    Sqrt = mybir.ActivationFunctionType.Sqrt
    Alu = mybir.AluOpType
    AxX = mybir.AxisListType.X

    nc = bacc.Bacc("TRN2", target_bir_lowering=False, debug=False,
                   num_devices=NCORES)

    def din(name, shape, dt):
        return nc.dram_tensor(name, shape, dt, kind="ExternalInput").ap()

    hT = din("hT", [D, S], bf16)          # h[b].T
    hrows = din("hrows", [SQ, D], f32)    # h[b, s_rows] (residual, fp32)
    rhT = din("rhT", [RL, S], bf16)       # rh[b].T
    rhTq = din("rhTq", [RL, SQ], bf16)    # rh[b, s_rows].T
    Wr_d = din("Wr", [D, D], bf16)
    Wrs_d = din("Wrs", [RL, D], bf16)     # cols already head-major
    Wrt_d = din("Wrt", [RL, D], bf16)
    Wf_d = din("Wf", [D, D], bf16)
    arv = din("arv", [D], f32)            # ar tiled per head
    g_d = din("g", [D], f32)
    b_d = din("b", [D], f32)
    out = nc.dram_tensor("out", [SQ, D], f32, kind="ExternalOutput").ap()
    rec_dram = nc.dram_tensor("rec_scratch", [H, SQ], f32)  # Internal
    den_dram = nc.dram_tensor("den_scratch", [H, SQ], f32)  # Internal

    def bcast_ap(src_ap, parts, free):
        return bass.AP(tensor=src_ap.tensor, offset=src_ap.offset,
                       ap=[[0, parts], [1, free]])

    with tile.TileContext(nc) as tc:
        ex = tc.nc  # same nc
        with tc.tile_pool(name="singles", bufs=1) as singles:
            # ---- constants / weights ----
            Wr_sb = singles.tile([128, DK, D], bf16)
            nc.sync.dma_start(out=Wr_sb,
                              in_=Wr_d.rearrange("(k p) n -> p k n", p=128))
            Wf_sb = singles.tile([128, DK, D], bf16)
            nc.sync.dma_start(out=Wf_sb,
                              in_=Wf_d.rearrange("(k p) n -> p k n", p=128))
            Wrs_sb = singles.tile([RL, D], bf16)
            nc.sync.dma_start(out=Wrs_sb, in_=Wrs_d)
            Wrt_sb = singles.tile([RL, D], bf16)
            nc.sync.dma_start(out=Wrt_sb, in_=Wrt_d)
            ar_bc = singles.tile([128, D], f32)
            nc.gpsimd.dma_start(out=ar_bc, in_=bcast_ap(arv, 128, D))
            g_bc = singles.tile([128, D], f32)
            nc.gpsimd.dma_start(out=g_bc, in_=bcast_ap(g_d, 128, D))
            b_bc = singles.tile([128, D], f32)
            nc.gpsimd.dma_start(out=b_bc, in_=bcast_ap(b_d, 128, D))
            eps_t = singles.tile([128, 1], f32)
            nc.vector.memset(eps_t, LN_EPS)

            rhT_sb = singles.tile([RL, S], bf16)
            nc.sync.dma_start(out=rhT_sb, in_=rhT)
            rhTq_sb = singles.tile([RL, SQ], bf16)
            nc.sync.dma_start(out=rhTq_sb, in_=rhTq)

            # frp: [t-tile, ti, (head: 64 fr cols + ones col)] bf16
            frp = singles.tile([128, KT, H * (HD + 1)], bf16)
            for h in range(H):
                nc.vector.memset(frp[:, :, h * (HD + 1) + HD:h * (HD + 1) + HD + 1], 1.0)
            sr_all = singles.tile([128, KT, H], f32)

            hT_v = hT.rearrange("(k p) t -> k p t", p=128)

            # ============ S2 first: rq / rkT (feeds S3 earliest) ============
            with tc.tile_pool(name="hTp", bufs=DK) as hTp, \
                 tc.tile_pool(name="s1tmp", bufs=3) as s1tmp:
                ps_a_cm = tc.tile_pool(name="ps_a", bufs=2, space="PSUM")
                ps_a = ps_a_cm.__enter__()
                rq_sb, rkT_sb = [None] * (H // 2), [None] * (H // 2)

                def emit_pair(j):
                    rq = hTp.tile([128, S], bf16, tag="rq", name=f"rq{j}")
                    for n in range(S // 512):
                        ps = ps_a.tile([128, 512], f32, tag="ps", name="ps")
                        nc.tensor.matmul(ps, lhsT=Wrt_sb[:, 128 * j:128 * (j + 1)],
                                         rhs=rhT_sb[:, 512 * n:512 * (n + 1)],
                                         start=True, stop=True)
                        nc.scalar.copy(out=rq[:, 512 * n:512 * (n + 1)], in_=ps)
                    rq_sb[j] = rq
                    rk = hTp.tile([128, SQ], bf16, tag="rk", name=f"rk{j}")
                    for n in range(NCH):
                        ps = ps_a.tile([128, 512], f32, tag="ps", name="ps")
                        nc.tensor.matmul(ps, lhsT=Wrs_sb[:, 128 * j:128 * (j + 1)],
                                         rhs=rhTq_sb[:, 512 * n:512 * (n + 1)],
                                         start=True, stop=True)
                        nc.scalar.copy(out=rk[:, 512 * n:512 * (n + 1)], in_=ps)
                    rkT_sb[j] = rk

                for j in range(H // 2):
                    emit_pair(j)

                # ================= S1: fr projection, frp, sr =================
                hT_sb = []
                for k in range(DK):
                    t = hTp.tile([128, S], bf16, tag="hT")
                    nc.sync.dma_start(out=t, in_=hT_v[k])
                    hT_sb.append(t)
                for i in range(KT):
                    ps = ps_a.tile([128, 512], f32, tag="ps")
                    for k in range(DK):
                        nc.tensor.matmul(ps, lhsT=hT_sb[k][:, 128 * i:128 * (i + 1)],
                                         rhs=Wr_sb[:, k, :],
                                         start=(k == 0), stop=(k == DK - 1))
                    nc.scalar.copy(
                        out=frp[:, i, :].rearrange("p (h c) -> p h c", c=HD + 1)[:, :, 0:HD],
                        in_=ps.rearrange("p (h c) -> p h c", c=HD))
                    lk = s1tmp.tile([128, 8, HD], f32, tag="lk")
                    frp_i = frp[:, i, :].rearrange("p (h c) -> p h c", c=HD + 1)[:, :, 0:HD]
                    nc.vector.scalar_tensor_tensor(
                        out=lk, in0=frp_i, scalar=SLOPE, in1=frp_i,
                        op0=Alu.mult, op1=Alu.max)
                    lka = s1tmp.tile([128, 8, HD], f32, tag="lka")
                    nc.vector.tensor_mul(lka, lk,
                                         ar_bc.rearrange("p (h c) -> p h c", c=HD))
                    nc.vector.reduce_sum(out=sr_all[:, i, :], in_=lka, axis=AxX)

                ps_a_cm.__exit__(None, None, None)

                # ===== S3: per-head attention; normalize folded in per pair =====
                ctx_sb = []
                for j in range(DK):
                    ctile = singles.tile([128, SQ], f32, tag=f"ctx{j}")
                    ctx_sb.append(ctile)
                hsa_sb = [None] * DK
                with tc.tile_pool(name="ps_bil", bufs=3, space="PSUM") as ps_bil, \
                     tc.tile_pool(name="ps_ctx", bufs=2, space="PSUM") as ps_ctx, \
                     tc.tile_pool(name="dstage", bufs=3) as dstage, \
                     tc.tile_pool(name="dpp", bufs=2) as dpp, \
                     tc.tile_pool(name="recb", bufs=2) as recbp, \
                     tc.tile_pool(name="pp", bufs=17) as pp:
                    for h in range(H):
                        j, off = h // 2, 64 * (h % 2)
                        ptiles = []
                        for ti in range(KT):
                            psb = ps_bil.tile([128, SQ], f32, tag="bil")
                            for c in range(NCH):
                                nc.tensor.matmul(
                                    psb[:, 512 * c:512 * (c + 1)],
                                    lhsT=rq_sb[j][off:off + 64, 128 * ti:128 * (ti + 1)],
                                    rhs=rkT_sb[j][off:off + 64, 512 * c:512 * (c + 1)],
                                    start=True, stop=True)
                            pt = pp.tile([128, SQ], bf16, tag="pt")
                            nc.scalar.activation(out=pt, in_=psb, func=Exp,
                                                 bias=sr_all[:, ti, h:h + 1])
                            ptiles.append(pt)
                        for c in range(NCH):
                            psc = ps_ctx.tile([HD + 1, 512], f32, tag="ctxps")
                            for ti in range(KT):
                                nc.tensor.matmul(
                                    psc,
                                    lhsT=frp[:, ti, (HD + 1) * h:(HD + 1) * (h + 1)],
                                    rhs=ptiles[ti][:, 512 * c:512 * (c + 1)],
                                    start=(ti == 0), stop=(ti == KT - 1))
                            nc.vector.tensor_copy(
                                out=ctx_sb[j][off:off + 64, 512 * c:512 * (c + 1)],
                                in_=psc[0:HD, :])
                            dst = dstage.tile([1, 512], f32, tag="dst")
                            nc.vector.tensor_copy(out=dst, in_=psc[HD:HD + 1, :])
                            nc.sync.dma_start(
                                out=den_dram.ap()[h, 512 * c:512 * (c + 1)][None, :],
                                in_=dst)
                        if h % 2 == 1:
                            # normalize this d-tile pair inside the S3 shadow
                            dp = dpp.tile([8, 256], f32, tag="dp")
                            nc.sync.dma_start(
                                out=dp,
                                in_=den_dram.ap().rearrange("a q -> (a q)")[
                                    2 * j * SQ:(2 * j + 2) * SQ].rearrange(
                                        "(p c) -> p c", p=8))
                            rp = dpp.tile([8, 256], f32, tag="rp")
                            nc.vector.reciprocal(rp, dp)
                            nc.sync.dma_start(
                                out=rec_dram.ap().rearrange("a q -> (a q)")[
                                    2 * j * SQ:(2 * j + 2) * SQ].rearrange(
                                        "(p c) -> p c", p=8),
                                in_=rp)
                            recb = recbp.tile([128, SQ], f32, tag="recb")
                            for u in range(2):
                                nc.gpsimd.dma_start(
                                    out=recb[64 * u:64 * (u + 1), :],
                                    in_=bass.AP(tensor=rec_dram,
                                                offset=(2 * j + u) * SQ,
                                                ap=[[0, 64], [1, SQ]]))
                            hsa = hTp.tile([128, SQ], bf16, tag="hT")
                            nc.vector.tensor_mul(hsa, ctx_sb[j], recb)
                            hsa_sb[j] = hsa

                # ================= S4: fh + LN =================
                hrows_v = hrows.rearrange("(m p) d -> m p d", p=128)
                out_v = out.rearrange("(m p) d -> m p d", p=128)
                with tc.tile_pool(name="ps_fh", bufs=2, space="PSUM") as ps_fh, \
                     tc.tile_pool(name="lnp", bufs=6) as lnp, \
                     tc.tile_pool(name="hrp", bufs=3) as hrp:
                    for mi in range(MQ):
                        psf = ps_fh.tile([128, 512], f32, tag="fh")
                        for k in range(DK):
                            nc.tensor.matmul(psf,
                                             lhsT=hsa_sb[k][:, 128 * mi:128 * (mi + 1)],
                                             rhs=Wf_sb[:, k, :],
                                             start=(k == 0), stop=(k == DK - 1))
                        hr = hrp.tile([128, D], f32, tag="hr")
                        nc.sync.dma_start(out=hr, in_=hrows_v[mi])
                        xs = lnp.tile([128, D], f32, tag="xs")
                        nc.vector.tensor_add(xs, psf, hr)
                        stats = lnp.tile([128, 6], f32, tag="stats")
                        nc.vector.bn_stats(stats, xs)
                        mv = lnp.tile([128, 2], f32, tag="mv")
                        nc.vector.bn_aggr(mv, stats)
                        std = lnp.tile([128, 1], f32, tag="std")
                        nc.scalar.activation(out=std, in_=mv[:, 1:2], func=Sqrt,
                                             bias=eps_t)
                        rstd = lnp.tile([128, 1], f32, tag="rstd")
                        nc.vector.reciprocal(rstd, std)
                        xn = lnp.tile([128, D], f32, tag="xn")
                        nc.vector.tensor_scalar(out=xn, in0=xs,
                                                scalar1=mv[:, 0:1], scalar2=rstd,
                                                op0=Alu.subtract, op1=Alu.mult)
                        xo = lnp.tile([128, D], f32, tag="xo")
                        nc.vector.scalar_tensor_tensor(out=xo, in0=xn, scalar=1.0,
                                                       in1=g_bc, op0=Alu.mult,
                                                       op1=Alu.mult)
                        nc.vector.tensor_add(xo, xo, b_bc)
                        nc.sync.dma_start(out=out_v[mi], in_=xo)

    nc.compile()
    return nc


def _get_nc():
    if "nc" not in _CACHE:
        _CACHE["nc"] = _build()
    return _CACHE["nc"]


def _in_maps(h, rh, Wr, ar, Wrs, Wrt, Wf, ln_g, ln_b):
    h = np.asarray(h, np.float32)
    rh = np.asarray(rh, np.float32)
    in_maps = []
    for c in range(NCORES):
        b, q = c // 2, c % 2
        sl = slice(q * SQ, (q + 1) * SQ)
        in_maps.append({
            "hT": np.ascontiguousarray(h[b].T).astype(BF16),
            "hrows": np.ascontiguousarray(h[b, sl]),
            "rhT": np.ascontiguousarray(rh[b].T).astype(BF16),
            "rhTq": np.ascontiguousarray(rh[b, sl].T).astype(BF16),
            "Wr": np.asarray(Wr, np.float32).astype(BF16),
            "Wrs": np.asarray(Wrs, np.float32).astype(BF16),
            "Wrt": np.asarray(Wrt, np.float32).astype(BF16),
            "Wf": np.asarray(Wf, np.float32).astype(BF16),
            "arv": np.ascontiguousarray(np.tile(np.asarray(ar, np.float32), H)),
            "g": np.asarray(ln_g, np.float32),
            "b": np.asarray(ln_b, np.float32),
        })
    return in_maps


def _assemble(results):
    outp = np.empty((B, S, D), np.float32)
    for c in range(NCORES):
        b, q = c // 2, c % 2
        outp[b, q * SQ:(q + 1) * SQ] = results[c]["out"]
    return outp


def kernel(h, rh, Wl, Wr, al, ar, Wrs, Wrt, Wf, ln_g, ln_b, **_ignored):
    nc = _get_nc()
    from concourse.bass_utils import run_bass_kernel_spmd

    in_maps = _in_maps(h, rh, Wr, ar, Wrs, Wrt, Wf, ln_g, ln_b)
    res = run_bass_kernel_spmd(nc, in_maps, core_ids=list(range(NCORES)))
    _CACHE["last_results"] = res
    return _assemble(res.results)
